# revision 1
# baseline (speedup 1.0000x reference)
"""CRF negative-log-likelihood loss kernel for Trainium2 (Bass/Tile).

Strategy (data-parallel over batch, 8 NeuronCores, 32 rows each):
  - log-partition via probability-domain forward scan:
        a_t = exp(x_t - c) * (E^T a_{t-1}),   E = exp(trans)
    with a FIXED per-step rescale constant c (exp bias), so the cumulative
    log-offset is exactly c*L per row -- no data-dependent rescale machinery.
  - logits are cast to bf16 on host; one xbar DMA transpose produces the
    full exp-input in [k, t, b] layout (zero PE/DVE transposes). exp runs
    on the Scalar engine in [128, *] chunks.
  - the scan is split into CHAINS independent column chains to pipeline the
    PE->DVE->PE latency. The PE's stationary (E) is loaded once; subsequent
    scan matmuls set ldweights=False and reuse the loaded array.
  - full a-history kept in SBUF (bf16); logZ[b] recovered at t=seq_len[b]-1
    by gpsimd ap_gather column gathers + Ln;  logZ += c*L[b].
  - gold score:
      unary  = gather logits[b,t,lab] from a (b,j)-partitioned bf16 copy
               (one gpsimd ap_gather) + one masked tensor_tensor_reduce.
      pair   = trans[lab_t,lab_{t+1}] via ap_gather from a replicated flat
               bf16 trans; mask folded into the gather index.
  - per-core partial losses summed on host.
"""

import os

import numpy as np

STAGE = int(os.environ.get("KSTAGE", "9"))

B, T, K = 256, 512, 128
NCORES = 8
BL = B // NCORES          # 32 batch rows per core
NJ = 4                    # t-chunks for the score layout: partition=(b, j)
TJ = T // NJ              # 128
C_LOG = 5.9               # fixed per-step log rescale (exp bias)
CHAINS = 2                # independent scan column chains
LDW_REUSE = True          # reuse PE stationary across scan matmuls
INPLACE_EXP = True        # exp writes over the transposed tile
CW = BL // CHAINS         # columns per chain
ECH = 16                  # steps per exp chunk

_CACHE = {}


# ----------------------------------------------------------------------------
# host-side constant tables (shape-only, input independent)
# ----------------------------------------------------------------------------
def _host_consts():
    c = {}
    c["c_id32"] = np.eye(32, dtype=np.float32)
    # unary combine: [128, 32] sel[p, b] = (p//4 == b)
    c["c_comb_u"] = (np.arange(128)[:, None] // NJ == np.arange(32)[None, :]).astype(
        np.float32
    )
    # pair combine, one per b_local: [128, 32] sel[p,b] = (p%16==0 and b==4*(p//16)+bl)
    for bl in range(4):
        m = (np.arange(128)[:, None] % 16 == 0) & (
            np.arange(32)[None, :] == 4 * (np.arange(128)[:, None] // 16) + bl
        )
        c[f"c_comb_p{bl}"] = m.astype(np.float32)
    c["c_onescol"] = np.ones((128, 1), dtype=np.float32)
    return c


# ----------------------------------------------------------------------------
# the Tile program
# ----------------------------------------------------------------------------
def _build_program():
    from contextlib import ExitStack

    import concourse.bass as bass
    import concourse.mybir as mybir
    import concourse.tile as tile
    from concourse import bacc

    f32 = mybir.dt.float32
    bf16 = mybir.dt.bfloat16
    i32 = mybir.dt.int32
    i16 = mybir.dt.int16
    AX = mybir.AxisListType
    OP = mybir.AluOpType
    ACTF = mybir.ActivationFunctionType

    nc = bacc.Bacc("TRN2", target_bir_lowering=False, debug=False)

    logits_d = nc.dram_tensor("logits_bf", [BL, T * K], bf16, kind="ExternalInput").ap()
    seq_d = nc.dram_tensor("seq_lens", [BL], i32, kind="ExternalInput").ap()
    idx16_d = nc.dram_tensor("idx16", [128, 264], i16, kind="ExternalInput").ap()
    m2eo_d = nc.dram_tensor("m2eo", [128, 32 * TJ], bf16, kind="ExternalInput").ap()
    trans_d = nc.dram_tensor("trans", [K, K], f32, kind="ExternalInput").ap()
    transf0_d = nc.dram_tensor("trans_f0", [K * K + 1], f32, kind="ExternalInput").ap()
    consts_np = _host_consts()
    cd = {}
    for name, arr in consts_np.items():
        cd[name] = nc.dram_tensor(
            name, list(arr.shape), mybir.dt.from_np(arr.dtype), kind="ExternalInput"
        ).ap()
    loss_d = nc.dram_tensor("loss", [1, 1], f32, kind="ExternalOutput").ap()

    with tile.TileContext(nc) as tc, ExitStack() as ctx:
        const_pool = ctx.enter_context(tc.tile_pool(name="const", bufs=1))
        big_pool = ctx.enter_context(tc.tile_pool(name="big", bufs=1))
        small_pool = ctx.enter_context(tc.tile_pool(name="small", bufs=1))
        ps_ch = [
            ctx.enter_context(tc.tile_pool(name=f"ps{c}", bufs=3, space="PSUM"))
            for c in range(CHAINS)
        ]
        ps_misc = ctx.enter_context(tc.tile_pool(name="ps_misc", bufs=1, space="PSUM"))

        def load_const(name, eng=None):
            ap = cd[name]
            t = const_pool.tile(list(ap.shape), ap.dtype, tag=name)
            (eng or nc.gpsimd).dma_start(t[:], ap[:])
            return t

        # ---- small input DMAs first on the sync queue, then transposes ----
        trs = small_pool.tile([K, K], f32, tag="trs")
        nc.sync.dma_start(trs[:], trans_d[:, :])
        seq32 = small_pool.tile([BL, 1], i32, tag="seq32")
        nc.sync.dma_start(seq32[:], seq_d.rearrange("(b o) -> b o", o=1))
        e_bf = const_pool.tile([K, K], bf16, tag="e_bf")
        nc.scalar.activation(e_bf[:], trs[:], ACTF.Exp)

        # ---- the full transposed logits [k, t, b] via xbar DMA ----
        exraw = big_pool.tile([128, T * BL], bf16, tag="exraw")

        def emit_transpose(q):
            nc.sync.dma_start_transpose(
                exraw[:, q * 4096 : (q + 1) * 4096].rearrange(
                    "k (t b) -> k t b", t=128
                ),
                logits_d[:, q * 16384 : (q + 1) * 16384],
            )

        emit_transpose(0)

        # ---- host-precomputed gather/mask tables (loaded inside the scan) ----
        data_u = big_pool.tile([128, TJ * K], bf16, tag="data_u")
        src_u = bass.AP(logits_d.tensor, 0, [[TJ * K, 128], [1, TJ * K]])
        tr_rep = big_pool.tile([128, K * K + 1], f32, tag="tr_rep")
        idx16 = small_pool.tile([128, 264], i16, tag="idx16")
        pidx16 = idx16[:, 0:128]
        gi16 = idx16[:, 128:256]
        cap_iq = [idx16[:, 256 + 2 * q : 258 + 2 * q] for q in range(4)]
        m2eo = small_pool.tile([128, 32 * TJ], bf16, tag="m2eo")
        m2e = m2eo[:, 0 : 16 * TJ]
        m2o = m2eo[:, 16 * TJ : 32 * TJ]
        id32 = const_pool.tile([32, 32], f32, tag="c_id32")
        onescol_f = const_pool.tile([128, 1], f32, tag="c_onescol")
        onescol_bf = const_pool.tile([128, 1], bf16, tag="onescol_bf16")
        seqf = small_pool.tile([BL, 1], f32, tag="seqf")

        def emit_tables():
            nc.gpsimd.dma_start(idx16[:], idx16_d[:, :])
            nc.gpsimd.dma_start(m2eo[:], m2eo_d[:, :])
            nc.gpsimd.dma_start(id32[:], cd["c_id32"][:])
            nc.gpsimd.dma_start(onescol_f[:], cd["c_onescol"][:])
            nc.gpsimd.tensor_copy(onescol_bf[:], onescol_f[:])
            nc.gpsimd.tensor_copy(seqf[:], seq32[:])

        # ================= exp chunks + the scan =================
        if INPLACE_EXP:
            exe = exraw
        else:
            exe = big_pool.tile([128, T * BL], bf16, tag="exe")
        NCH_E = T // ECH
        bias_c = small_pool.tile([128, 1], f32, tag="bias_c")
        nc.vector.memset(bias_c[:], -C_LOG)

        def emit_exp(m):
            sl = slice(m * ECH * BL, (m + 1) * ECH * BL)
            nc.scalar.activation(exe[:, sl], exraw[:, sl], ACTF.Exp, bias=bias_c[:])

        emit_exp(0)
        emit_exp(1)
        emit_exp(2)

        a_hist = big_pool.tile([128, T * BL], bf16, tag="a_hist")
        nc.vector.tensor_copy(a_hist[:, 0:BL], exe[:, 0:BL])

        first_mm = True
        for t in range(1, T if STAGE >= 2 else 2):
            if t == 2:
                emit_transpose(1)
            if t == 4:
                emit_tables()
            if t == 18:
                emit_transpose(2)
            if t == 34:
                emit_transpose(3)
            if t % ECH == 0:
                m = t // ECH + 2
                if m < NCH_E:
                    emit_exp(m)
            if 24 <= t < 56 and t % 4 == 0:
                r = (t - 24) // 4
                n = 2048 if r < 7 else 2049
                sl = slice(r * 2048, r * 2048 + n)
                nc.gpsimd.dma_start(
                    tr_rep[:, sl],
                    bass.AP(transf0_d.tensor, sl.start, [[0, 128], [1, n]]),
                )
            if 6 <= t < 22 and t % 4 == 2:
                r = (t - 6) // 4
                sl = slice(r * 4096, (r + 1) * 4096)
                nc.gpsimd.dma_start(data_u[:, sl], src_u[:, sl])

            for c in range(CHAINS):
                lo = c * CW
                hi = (c + 1) * CW
                up = ps_ch[c].tile([K, CW], f32, tag=f"u{c}")
                mm = nc.tensor.matmul(
                    up[:],
                    e_bf[:],
                    a_hist[:, (t - 1) * BL + lo : (t - 1) * BL + hi],
                    start=True,
                    stop=True,
                )
                if LDW_REUSE and not first_mm:
                    mm.ins.ldweights = False
                first_mm = False
                nc.vector.tensor_mul(
                    a_hist[:, t * BL + lo : t * BL + hi], up[:], exe[:, t * BL + lo : t * BL + hi]
                )

        if STAGE < 3:
            dummy = small_pool.tile([1, 1], f32, tag="dummy")
            nc.vector.tensor_reduce(dummy[:], a_hist[0:1, (T-1)*BL:T*BL] if STAGE >= 2 else exe[0:1, 0:BL], AX.X, OP.add)
            nc.sync.dma_start(loss_d[:, :], dummy[:])
            nc.compile()
            return nc, consts_np
        ctx.enter_context(tc.high_priority(offset=-(10 ** 6)))
        # ================= gold score =================
        # unary: gather logits[b,t,lab] then masked sum
        u_acc8 = small_pool.tile([128, 8], f32, tag="u_acc8")
        junk5 = small_pool.tile([128, 512], f32, tag="junk5")
        gch = []
        for i in range(2):
            gchi = small_pool.tile([128, 1024], bf16, tag=f"gch{i}", name=f"gch{i}")
            gch.append(gchi)
        for q in range(4):
            g = gch[q % 2]
            nc.gpsimd.ap_gather(
                g[:], data_u[:], gi16[:, q * 32 : (q + 1) * 32], channels=128,
                num_elems=TJ * K // 2, d=2, num_idxs=512,
            )
            gv = g[:].rearrange("p (i two) -> p i two", two=2)
            nc.vector.scalar_tensor_tensor(
                junk5[:], gv[:, :, 0:1], 1.0, m2e[:, q * 512 : (q + 1) * 512],
                OP.mult, OP.mult, accum_out=u_acc8[:, 2 * q : 2 * q + 1],
            )
            nc.vector.scalar_tensor_tensor(
                junk5[:], gv[:, :, 1:2], 1.0, m2o[:, q * 512 : (q + 1) * 512],
                OP.mult, OP.mult, accum_out=u_acc8[:, 2 * q + 1 : 2 * q + 2],
            )
        u_part = small_pool.tile([128, 1], f32, tag="u_part")
        nc.vector.tensor_reduce(u_part[:], u_acc8[:], AX.X, OP.add)

        # pair: gather + 4 reduces
        pair_g = small_pool.tile([128, 2048], f32, tag="pair_g")
        nc.gpsimd.ap_gather(
            pair_g[:], tr_rep[:], pidx16, channels=128, num_elems=K * K + 1, d=1,
            num_idxs=2048,
        )
        pair_p = small_pool.tile([128, 4], f32, tag="pair_p")
        for q in range(4):
            nc.vector.tensor_reduce(
                pair_p[:, q : q + 1], pair_g[:, q * 512 : (q + 1) * 512], AX.X, OP.add
            )

        comb_u = load_const("c_comb_u")
        comb_p = [load_const(f"c_comb_p{bl}") for bl in range(4)]

        # score[b] accumulated in one [32,1] psum
        score_ps = ps_misc.tile([32, 1], f32, tag="mm_score")
        nc.tensor.matmul(score_ps[:], comb_u[:], u_part[:], start=True, stop=False)
        for bl in range(4):
            nc.tensor.matmul(
                score_ps[:],
                comb_p[bl][:],
                pair_p[:, bl : bl + 1],
                start=False,
                stop=(bl == 3),
            )
        score_sb = small_pool.tile([32, 1], f32, tag="score_sb")
        nc.vector.tensor_copy(score_sb[:], score_ps[:])

        if STAGE < 4:
            dummy = small_pool.tile([1, 1], f32, tag="dummy")
            nc.vector.tensor_reduce(dummy[:], u_part[0:1, :], AX.X, OP.add)
            nc.sync.dma_start(loss_d[:, :], dummy[:])
            nc.compile()
            return nc, consts_np
        # ================= capture logZ =================
        acap_q = []
        for q in range(4):
            aq = small_pool.tile([128, 64], bf16, tag=f"acap{q}")
            nc.gpsimd.ap_gather(
                aq[:], a_hist[:, q * 4096 : (q + 1) * 4096], cap_iq[q],
                channels=128, num_elems=2048, d=2, num_idxs=32,
            )
            acap_q.append(aq)
        lrow_ps = ps_misc.tile([1, 32], f32, tag="mm_small")
        nc.tensor.matmul(lrow_ps[:], seqf[:], id32[:], start=True, stop=True)
        lrow = small_pool.tile([1, 32], f32, tag="lrow")
        nc.vector.tensor_copy(lrow[:], lrow_ps[:])
        lm1row = small_pool.tile([1, 32], f32, tag="lm1row")
        nc.vector.tensor_scalar(lm1row[:], lrow[:], -1.0, None, OP.add)
        sig = small_pool.tile([1, BL], f32, tag="sig")
        nc.vector.memset(sig[:], 0.0)
        mq = small_pool.tile([1, BL], f32, tag="mq")
        mq2 = small_pool.tile([1, BL], f32, tag="mq2")
        sq = small_pool.tile([1, BL], f32, tag="sq")
        for q in range(4):
            sg_ev = ps_misc.tile([1, 16], f32, tag="mm_small")
            nc.tensor.matmul(
                sg_ev[:], onescol_bf[:], acap_q[q][:, 0:64:4], start=True, stop=True
            )
            sg_od = ps_misc.tile([1, 16], f32, tag="mm_score")
            nc.tensor.matmul(
                sg_od[:], onescol_bf[:], acap_q[q][:, 3:64:4], start=True, stop=True
            )
            nc.vector.tensor_copy(sq[:, 0:32:2], sg_ev[:])
            nc.vector.tensor_copy(sq[:, 1:32:2], sg_od[:])
            nc.vector.tensor_scalar(mq[:], lm1row[:], float(q * 128), None, OP.is_ge)
            nc.vector.tensor_scalar(
                mq2[:], lm1row[:], float((q + 1) * 128), None, OP.is_lt
            )
            nc.vector.tensor_tensor(mq[:], mq[:], mq2[:], OP.mult)
            nc.vector.tensor_tensor(sq[:], sq[:], mq[:], OP.mult)
            nc.vector.tensor_tensor(sig[:], sig[:], sq[:], OP.add)
        lz = small_pool.tile([1, BL], f32, tag="lz")
        nc.scalar.activation(lz[:], sig[:], ACTF.Ln)
        # logZ = ln(sig) + c*L
        lzm = small_pool.tile([1, BL], f32, tag="lzm")
        nc.vector.scalar_tensor_tensor(
            lzm[:], lrow[:], C_LOG, lz[:], OP.mult, OP.add
        )

        # ================= final loss =================
        scT_ps = ps_misc.tile([1, 32], f32, tag="mm_small")
        nc.tensor.matmul(scT_ps[:], score_sb[:], id32[:], start=True, stop=True)
        diff = small_pool.tile([1, BL], f32, tag="diff")
        nc.vector.tensor_sub(diff[:], lzm[:], scT_ps[:])
        loss_sb = small_pool.tile([1, 1], f32, tag="loss_sb")
        nc.vector.tensor_reduce(loss_sb[:], diff[:], AX.X, OP.add)
        nc.sync.dma_start(loss_d[:, :], loss_sb[:])

    nc.compile()
    return nc, consts_np


def _get_program():
    if "prog" not in _CACHE:
        _CACHE["prog"] = _build_program()
    return _CACHE["prog"]


def _core_tables(lab, L):
    """Index/mask tables derived from labels [BL,T] and seq_lens [BL]."""
    import ml_dtypes

    t = {}
    lf = lab.reshape(-1)
    lab_w = lf.reshape(8, 128, 16).transpose(0, 2, 1).reshape(128, 128)
    lfn = np.append(lf[1:], 0)
    labn_w = lfn.reshape(8, 128, 16).transpose(0, 2, 1).reshape(128, 128)
    p = np.arange(128)[:, None]
    sgrid = np.arange(128)[None, :]
    i_seq = sgrid * 16 + (p % 16)
    tw = i_seq % 512
    bw = 4 * (p // 16) + i_seq // 512
    m_w = (tw + 1) < L[bw]
    pidx = lab_w * 128 + labn_w
    pidx16 = ((pidx - 16384) * m_w + 16384).astype(np.int16)
    # unary: lab4[p, tl] = lab[b, j*128+tl], p = 4b+j
    lab4 = lab.reshape(BL, NJ, TJ).reshape(128, TJ)
    tl = np.arange(TJ)[None, :]
    gi16 = ((tl * K + lab4) >> 1).astype(np.int16)
    b_of_p = np.arange(128) // NJ
    j_of_p = np.arange(128) % NJ
    mask_u = (j_of_p[:, None] * TJ + tl) < L[b_of_p][:, None]
    par = (lab4 & 1).astype(np.float32)
    mp_o = mask_u * par
    mp_e = mask_u - mp_o
    diag = (p % 16 == np.arange(16)[None, :]).astype(np.float32)  # [128,16]
    t["m2e"] = (mp_e[:, :, None] * diag[:, None, :]).reshape(128, 16 * TJ).astype(
        ml_dtypes.bfloat16
    )
    t["m2o"] = (mp_o[:, :, None] * diag[:, None, :]).reshape(128, 16 * TJ).astype(
        ml_dtypes.bfloat16
    )
    # capture: idxp[p, c] = (L[c*16+p%16]-1)*16 + (c*16+p%16)//2
    cgrid = np.arange(2)[None, :]
    bcap = cgrid * 16 + (p % 16)
    idxp = (L[bcap] - 1) * 16 + bcap // 2
    caps = [np.clip(idxp - q * 2048, 0, 2047).astype(np.int16) for q in range(4)]
    t["idx16"] = np.concatenate([pidx16, gi16] + caps, axis=1)
    t["m2eo"] = np.concatenate([t.pop("m2e"), t.pop("m2o")], axis=1)
    return t


def _make_in_maps(logits, labels, seq_lens, trans, consts_np):
    import ml_dtypes

    logits = np.asarray(logits, dtype=np.float32)
    labels = np.asarray(labels, dtype=np.int32)
    seq_lens = np.asarray(seq_lens, dtype=np.int32)
    trans = np.asarray(trans, dtype=np.float32)
    logits_bf = logits.reshape(B, T * K).astype(ml_dtypes.bfloat16)
    trans_f0 = np.append(trans.reshape(-1), np.float32(0)).astype(np.float32)

    in_maps = []
    for c in range(NCORES):
        sl = slice(c * BL, (c + 1) * BL)
        m = {
            "logits_bf": np.ascontiguousarray(logits_bf[sl]),
            "seq_lens": np.ascontiguousarray(seq_lens[sl]),
            "trans": trans,
            "trans_f0": trans_f0,
        }
        m.update(_core_tables(labels[sl], seq_lens[sl]))
        m.update(consts_np)
        in_maps.append(m)
    return in_maps


def kernel(logits, labels, seq_lens, trans):
    from concourse.bass_utils import run_bass_kernel_spmd

    nc, consts_np = _get_program()
    in_maps = _make_in_maps(logits, labels, seq_lens, trans, consts_np)
    res = run_bass_kernel_spmd(nc, in_maps, list(range(NCORES)))
    total = sum(float(res.results[c]["loss"][0, 0]) for c in range(NCORES))
    return np.float32(total)



# revision 10
# speedup vs baseline: 1.2481x; 1.2481x over previous
"""CRF negative-log-likelihood loss kernel for Trainium2 (Bass/Tile).

Strategy (data-parallel over batch, 8 NeuronCores, 32 rows each):
  - log-partition via probability-domain scans with a FIXED per-step rescale
    (exp bias c):  a_t = exp(x_t - c) * (E^T a_{t-1}),  E = exp(trans).
  - meet-in-the-middle: the recursion is linear, so
        Z_b = a_M[b] . w_{L_b-1-M}[b]
    where w is a BACKWARD recursion w_j = E (d_{L_b-j} * w_{j-1}), w_0 = 1.
    fwd runs t=1..256 and bwd j=1..255 as two INDEPENDENT serial chains that
    pipeline on PE/DVE -- half the serial depth of a single 511-step scan.
  - the bwd exp-table is per-row time-reversed ON HOST (pure layout gather of
    logits), so the device needs no masking; rows with L_b-1 <= M instead
    capture a at t=L_b-1 (then w_cap = w_0 = ones).  Uniformly:
        logZ_b = ln(a_hist[t_a] . w_hist[j_w]) + c*L_b,
        t_a = min(L_b-1, M),  j_w = max(L_b-1-M, 0).
  - gold score: only the per-core TOTAL is needed (loss is a sum), so
      unary = one ap_gather from the transposed raw-logits tile with
              per-gpsimd-core label bucketing + masked accumulation,
      pair  = ap_gather from a replicated flat trans (mask folded into idx).
    Their reductions run on the otherwise-idle GPSIMD engine.
  - per-core partial losses summed on host.
"""

import numpy as np

B, T, K = 256, 512, 128
NCORES = 8
BL = B // NCORES          # 32 batch rows per core
M = 256                   # fwd computes a_t for t=0..M  (256 serial steps)
JMAX = 255                # bwd computes w_j for j=0..JMAX (255 serial steps)
NTF = M + 1               # fwd time slots
NTB = JMAX                # bwd j slots (j=1..JMAX stored at slot j-1)
C_LOG = 5.9               # fixed per-step log rescale (exp bias)
NIU = 2560                # padded unary slots per gpsimd core (mean ~1024)

_CACHE = {}


def _build_program():
    from contextlib import ExitStack

    import concourse.bass as bass
    import concourse.mybir as mybir
    import concourse.tile as tile
    from concourse import bacc

    f32 = mybir.dt.float32
    bf16 = mybir.dt.bfloat16
    i16 = mybir.dt.int16
    AX = mybir.AxisListType
    OP = mybir.AluOpType
    ACTF = mybir.ActivationFunctionType

    nc = bacc.Bacc("TRN2", target_bir_lowering=False, debug=False)

    CF = NTF * BL             # 8224 fwd raw/exe cols
    CB = NTB * BL             # 8160 bwd raw/exe cols
    CW = (JMAX + 1) * BL      # 8192 w_hist cols

    raw_d = nc.dram_tensor("raw_all", [128, CF + CB], bf16, kind="ExternalInput").ap()
    trans_d = nc.dram_tensor("trans", [K, K], f32, kind="ExternalInput").ap()
    transT_d = nc.dram_tensor("transT", [K, K], f32, kind="ExternalInput").ap()
    transf0_d = nc.dram_tensor("trans_f0", [K * K + 1], f32, kind="ExternalInput").ap()
    seqf_d = nc.dram_tensor("seqf_row", [1, BL], f32, kind="ExternalInput").ap()
    idxcap_d = nc.dram_tensor("idx_cap", [128, 4], i16, kind="ExternalInput").ap()
    pidx_d = nc.dram_tensor("pidx", [128, 128], i16, kind="ExternalInput").ap()
    idxu_d = nc.dram_tensor("idx_u", [128, NIU // 16], i16, kind="ExternalInput").ap()
    mue_d = nc.dram_tensor("mu_e", [128, NIU], bf16, kind="ExternalInput").ap()
    muo_d = nc.dram_tensor("mu_o", [128, NIU], bf16, kind="ExternalInput").ap()
    loss_d = nc.dram_tensor("loss", [1, 1], f32, kind="ExternalOutput").ap()

    with tile.TileContext(nc) as tc, ExitStack() as ctx:
        big_pool = ctx.enter_context(tc.tile_pool(name="big", bufs=1))
        small_pool = ctx.enter_context(tc.tile_pool(name="small", bufs=1))
        ps_f = ctx.enter_context(tc.tile_pool(name="psf", bufs=3, space="PSUM"))
        ps_b = ctx.enter_context(tc.tile_pool(name="psb", bufs=3, space="PSUM"))
        ps_misc = ctx.enter_context(tc.tile_pool(name="ps_misc", bufs=1, space="PSUM"))

        # ---------------- SBUF tiles ----------------
        raw_all = big_pool.tile([128, CF + CB], bf16, tag="raw_all")
        exe_f = big_pool.tile([128, CF], bf16, tag="exe_f")
        exe_b = big_pool.tile([128, CB], bf16, tag="exe_b")
        a_hist = big_pool.tile([128, CF], bf16, tag="a_hist")
        w_hist = big_pool.tile([128, CW], bf16, tag="w_hist")
        tr_rep = big_pool.tile([128, K * K + 1], f32, tag="tr_rep")

        trs = small_pool.tile([K, K], f32, tag="trs")
        trsT = small_pool.tile([K, K], f32, tag="trsT")
        e_bf = small_pool.tile([K, K], bf16, tag="e_bf")
        et_bf = small_pool.tile([K, K], bf16, tag="et_bf")
        seqf = small_pool.tile([1, BL], f32, tag="seqf")
        idx_cap = small_pool.tile([128, 4], i16, tag="idx_cap")
        pidx = small_pool.tile([128, 128], i16, tag="pidx")
        idx_u = small_pool.tile([128, NIU // 16], i16, tag="idx_u")
        mu_e = small_pool.tile([128, NIU], bf16, tag="mu_e")
        mu_o = small_pool.tile([128, NIU], bf16, tag="mu_o")
        bias_c = small_pool.tile([128, 1], f32, tag="bias_c")
        ones_col = small_pool.tile([128, 1], bf16, tag="ones_col")

        gu = small_pool.tile([128, 2 * NIU], bf16, tag="gu")
        junk = small_pool.tile([128, NIU], bf16, tag="junk")
        u_acc = small_pool.tile([128, 2], f32, tag="u_acc")
        pair_g = small_pool.tile([128, 2048], f32, tag="pair_g")
        ga = small_pool.tile([128, 64], bf16, tag="ga")
        gw = small_pool.tile([128, 64], bf16, tag="gw")
        prod = small_pool.tile([128, 64], bf16, tag="prod")
        dots = small_pool.tile([1, BL], f32, tag="dots")
        ln_row = small_pool.tile([1, BL], f32, tag="ln_row")
        lc_row = small_pool.tile([1, BL], f32, tag="lc_row")
        t1 = small_pool.tile([1, 1], f32, tag="t1")
        loss_sb = small_pool.tile([1, 1], f32, tag="loss_sb")

        # ---------------- prologue ----------------
        # small inputs on the sync queue
        nc.sync.dma_start(trs[:], trans_d[:, :])
        nc.sync.dma_start(trsT[:], transT_d[:, :])
        nc.sync.dma_start(seqf[:], seqf_d[:, :])
        nc.sync.dma_start(idx_cap[:], idxcap_d[:, :])

        # raw logits: fwd part chunked on sync queue, bwd part on gpsimd queue
        FCH = [0, 1024, 3072, 5120, 7168, CF]
        BCH = [0, 1024, 3072, 5120, 7168, CB]

        def dma_f(i):
            nc.sync.dma_start(raw_all[:, FCH[i] : FCH[i + 1]], raw_d[:, FCH[i] : FCH[i + 1]])

        def dma_b(i):
            nc.gpsimd.dma_start(
                raw_all[:, CF + BCH[i] : CF + BCH[i + 1]],
                raw_d[:, CF + BCH[i] : CF + BCH[i + 1]],
            )

        dma_f(0)
        dma_b(0)

        # gather/mask tables + replicated trans on the gpsimd queue
        def emit_tables():
            nc.gpsimd.dma_start(pidx[:], pidx_d[:, :])
            nc.gpsimd.dma_start(idx_u[:], idxu_d[:, :])
            nc.gpsimd.dma_start(mu_e[:], mue_d[:, :])
            nc.gpsimd.dma_start(mu_o[:], muo_d[:, :])

        def emit_trrep(r):
            n = 2048 if r < 7 else 2049
            sl = slice(r * 2048, r * 2048 + n)
            nc.gpsimd.dma_start(
                tr_rep[:, sl], bass.AP(transf0_d.tensor, sl.start, [[0, 128], [1, n]])
            )

        nc.vector.memset(bias_c[:], -C_LOG)
        nc.vector.memset(ones_col[:], 1.0)
        nc.scalar.activation(e_bf[:], trs[:], ACTF.Exp)
        nc.scalar.activation(et_bf[:], trsT[:], ACTF.Exp)

        # exp chunks (scalar engine): 32 t-slots at a time
        def exp_f(k):
            c0, c1 = k * 1024, min((k + 1) * 1024, CF)
            nc.scalar.activation(exe_f[:, c0:c1], raw_all[:, c0:c1], ACTF.Exp, bias=bias_c[:])

        def exp_b(k):
            c0, c1 = k * 1024, min((k + 1) * 1024, CB)
            nc.scalar.activation(
                exe_b[:, c0:c1], raw_all[:, CF + c0 : CF + c1], ACTF.Exp, bias=bias_c[:]
            )

        exp_f(0)
        exp_b(0)

        # init states
        nc.vector.tensor_copy(a_hist[:, 0:BL], exe_f[:, 0:BL])
        nc.vector.memset(w_hist[:, 0:BL], 1.0)

        # ---------------- the two scans, interleaved ----------------
        for s in range(1, M + 1):
            if s in (2, 18, 34, 50):
                i = (s - 2) // 16 + 1
                dma_f(i)
                dma_b(i)
            if s == 66:
                emit_tables()
            if 70 <= s < 102 and (s - 70) % 4 == 0:
                emit_trrep((s - 70) // 4)
            if s % 32 == 8:
                k = s // 32 + 1
                if k * 1024 < CF:
                    exp_f(k)
            if s % 32 == 24:
                k = s // 32 + 1
                if k * 1024 < CB:
                    exp_b(k)

            # fwd step t=s:  a_s = exe_f[s] * (E^T a_{s-1})
            up_f = ps_f.tile([K, BL], f32, tag="up_f")
            nc.tensor.matmul(
                up_f[:], e_bf[:], a_hist[:, (s - 1) * BL : s * BL], start=True, stop=True
            )
            nc.vector.tensor_mul(
                a_hist[:, s * BL : (s + 1) * BL], up_f[:], exe_f[:, s * BL : (s + 1) * BL]
            )

            # bwd step j=s:  w_s = exe_b[s-1] * (E w_{s-1})
            if s <= JMAX:
                up_b = ps_b.tile([K, BL], f32, tag="up_b")
                nc.tensor.matmul(
                    up_b[:], et_bf[:], w_hist[:, (s - 1) * BL : s * BL], start=True, stop=True
                )
                nc.vector.tensor_mul(
                    w_hist[:, s * BL : (s + 1) * BL], up_b[:], exe_b[:, (s - 1) * BL : s * BL]
                )

        # ---------------- gold score (runs during the scan) ----------------
        ctx.enter_context(tc.high_priority(offset=-(10**6)))

        # unary: one bucketed gather from raw_all + masked accumulation (gpsimd)
        nc.gpsimd.ap_gather(
            gu[:], raw_all[:], idx_u[:, :], channels=128,
            num_elems=(CF + CB) // 2, d=2, num_idxs=NIU,
        )
        guv = gu[:].rearrange("p (i two) -> p i two", two=2)
        nc.vector.scalar_tensor_tensor(
            junk[:], guv[:, :, 0:1], 1.0, mu_e[:], OP.mult, OP.mult,
            accum_out=u_acc[:, 0:1],
        )
        nc.vector.scalar_tensor_tensor(
            junk[:], guv[:, :, 1:2], 1.0, mu_o[:], OP.mult, OP.mult,
            accum_out=u_acc[:, 1:2],
        )
        u_tot = small_pool.tile([1, 1], f32, tag="u_tot")
        nc.gpsimd.tensor_reduce(u_tot[:], u_acc[:], AX.XYZWC, OP.add)

        # pair: gather from replicated flat trans (idx 16384 -> 0.0 pad);
        # each gpsimd core's gather is replicated over its 16 partitions, so
        # the full reduce counts every pair 16x.
        nc.gpsimd.ap_gather(
            pair_g[:], tr_rep[:], pidx[:, :], channels=128,
            num_elems=K * K + 1, d=1, num_idxs=2048,
        )
        pair_tot = small_pool.tile([1, 1], f32, tag="pair_tot")
        nc.gpsimd.tensor_reduce(pair_tot[:], pair_g[:], AX.XYZWC, OP.add)
        score_tot = small_pool.tile([1, 1], f32, tag="score_tot")
        nc.vector.scalar_tensor_tensor(
            score_tot[:], pair_tot[:], 1.0 / 16.0, u_tot[:], OP.mult, OP.add
        )

        # ---------------- capture + logZ + loss ----------------
        nc.gpsimd.ap_gather(
            ga[:], a_hist[:], idx_cap[:, 0:2], channels=128,
            num_elems=CF // 2, d=2, num_idxs=32,
        )
        nc.gpsimd.ap_gather(
            gw[:], w_hist[:], idx_cap[:, 2:4], channels=128,
            num_elems=CW // 2, d=2, num_idxs=32,
        )
        nc.vector.tensor_mul(prod[:], ga[:], gw[:])
        dots_ev = ps_misc.tile([1, 16], f32, tag="mm_ev")
        nc.tensor.matmul(dots_ev[:], ones_col[:], prod[:, 0:64:4], start=True, stop=True)
        dots_od = ps_misc.tile([1, 16], f32, tag="mm_od")
        nc.tensor.matmul(dots_od[:], ones_col[:], prod[:, 3:64:4], start=True, stop=True)
        nc.vector.tensor_copy(dots[:, 0:BL:2], dots_ev[:])
        nc.vector.tensor_copy(dots[:, 1:BL:2], dots_od[:])
        nc.scalar.activation(ln_row[:], dots[:], ACTF.Ln)
        # lc = ln(dot) + c*L
        nc.vector.scalar_tensor_tensor(
            lc_row[:], seqf[:], C_LOG, ln_row[:], OP.mult, OP.add
        )
        nc.vector.tensor_reduce(t1[:], lc_row[:], AX.X, OP.add)
        nc.vector.tensor_sub(loss_sb[:], t1[:], score_tot[:])
        nc.sync.dma_start(loss_d[:, :], loss_sb[:])

    nc.compile()
    return nc


def _get_program():
    if "prog" not in _CACHE:
        _CACHE["prog"] = _build_program()
    return _CACHE["prog"]


def _core_tables(lgT_bf, lab, L):
    """Per-core tables: raw_all layout + gather indices/masks.

    lgT_bf: [K, T, BL] bf16 transposed logits, lab: [BL, T] int32, L: [BL]."""
    import ml_dtypes

    bf = ml_dtypes.bfloat16
    t = {}
    # raw_all: fwd t=0..M, then bwd j=1..JMAX time-reversed per row
    raw_f = lgT_bf[:, : M + 1, :].reshape(128, -1)
    tidx = np.maximum(L[None, :] - np.arange(1, JMAX + 1)[:, None], 0)  # [j, b]
    raw_b = lgT_bf[:, tidx, np.arange(BL)[None, :]].reshape(128, -1)
    t["raw_all"] = np.ascontiguousarray(
        np.concatenate([raw_f, raw_b], axis=1), dtype=bf
    )

    # capture indices (d=2 units): slot i=b lives at idx-col (c= b//16, pp=b%16)
    p = np.arange(128)[:, None]
    cgrid = np.arange(2)[None, :]
    bcap = cgrid * 16 + (p % 16)
    ta = np.minimum(L - 1, M)
    jw = np.maximum(L - 1 - M, 0)
    idx_a = (ta[bcap] * 16 + bcap // 2).astype(np.int16)
    idx_w = (jw[bcap] * 16 + bcap // 2).astype(np.int16)
    t["idx_cap"] = np.concatenate([idx_a, idx_w], axis=1)

    # pair idx: gpsimd core g handles rows 4g..4g+3, slot s=(col*16+pp) -> (r,tt)
    lab_n = np.concatenate([lab[:, 1:], np.zeros((BL, 1), np.int64)], axis=1)
    pid = lab.astype(np.int64) * 128 + lab_n  # value for pair (t, t+1)
    act = (np.arange(T)[None, :] + 1) < L[:, None]  # t+1 <= L-1
    pidv = np.where(act, pid, 16384)  # [BL, T]; slot t=511 always padded
    pidx = np.zeros((128, 128), np.int32)
    for g in range(8):
        rows = pidv[4 * g : 4 * g + 4].reshape(-1)  # [2048] slots r*512+tt
        s = np.arange(2048)
        pidx[16 * g + (s % 16), s // 16] = rows
    t["pidx"] = pidx.astype(np.int16)

    # unary: bucket active (b,t) entries by label's gpsimd core
    bb, tt = np.nonzero(np.arange(T)[None, :] < L[:, None])
    kk = lab[bb, tt]
    fwd_side = tt <= M
    unit = np.where(
        fwd_side, tt * 16 + bb // 2, (M + (L[bb] - tt)) * 16 + bb // 2
    ).astype(np.int64)
    par = (bb & 1).astype(np.int64)
    core = kk >> 4
    owner = kk & 15
    order = np.argsort(core, kind="stable")
    core_s, unit_s, owner_s, par_s = core[order], unit[order], owner[order], par[order]
    counts = np.bincount(core_s, minlength=8)
    assert counts.max() <= NIU, f"unary bucket overflow: {counts.max()}"
    idx_flat = np.zeros((8, NIU), np.int64)
    own_flat = np.full((8, NIU), -1, np.int64)
    par_flat = np.zeros((8, NIU), np.int64)
    off = 0
    for g in range(8):
        n = counts[g]
        idx_flat[g, :n] = unit_s[off : off + n]
        own_flat[g, :n] = owner_s[off : off + n]
        par_flat[g, :n] = par_s[off : off + n]
        off += n
    idx_u = np.zeros((128, NIU // 16), np.int16)
    s = np.arange(NIU)
    for g in range(8):
        idx_u[16 * g + (s % 16), s // 16] = idx_flat[g].astype(np.int16)
    t["idx_u"] = idx_u
    pp16 = np.arange(16)
    mu_e = np.zeros((128, NIU), np.float32)
    mu_o = np.zeros((128, NIU), np.float32)
    for g in range(8):
        own_match = own_flat[g][None, :] == pp16[:, None]  # [16, NIU]
        mu_e[16 * g : 16 * g + 16] = own_match & (par_flat[g][None, :] == 0)
        mu_o[16 * g : 16 * g + 16] = own_match & (par_flat[g][None, :] == 1)
    t["mu_e"] = mu_e.astype(bf)
    t["mu_o"] = mu_o.astype(bf)
    return t


def _make_in_maps(logits, labels, seq_lens, trans):
    import ml_dtypes

    bf = ml_dtypes.bfloat16
    logits = np.asarray(logits, dtype=np.float32)
    labels = np.asarray(labels, dtype=np.int64)
    seq_lens = np.asarray(seq_lens, dtype=np.int64)
    trans = np.asarray(trans, dtype=np.float32)
    trans_f0 = np.append(trans.reshape(-1), np.float32(0)).astype(np.float32)
    transT = np.ascontiguousarray(trans.T)

    in_maps = []
    for c in range(NCORES):
        sl = slice(c * BL, (c + 1) * BL)
        lgT_bf = logits[sl].transpose(2, 1, 0).astype(bf)  # [K, T, BL]
        L = seq_lens[sl]
        m = {
            "trans": trans,
            "transT": transT,
            "trans_f0": trans_f0,
            "seqf_row": L.astype(np.float32).reshape(1, BL),
        }
        m.update(_core_tables(lgT_bf, labels[sl], L))
        in_maps.append(m)
    return in_maps


def kernel(logits, labels, seq_lens, trans):
    from concourse.bass_utils import run_bass_kernel_spmd

    nc = _get_program()
    in_maps = _make_in_maps(logits, labels, seq_lens, trans)
    res = run_bass_kernel_spmd(nc, in_maps, list(range(NCORES)))
    total = sum(float(res.results[c]["loss"][0, 0]) for c in range(NCORES))
    return np.float32(total)


# revision 22
# speedup vs baseline: 1.4646x; 1.1734x over previous
"""CRF negative-log-likelihood loss kernel for Trainium2 (Bass/Tile).

Strategy (data-parallel over batch, 8 NeuronCores, 32 rows each):
  - log-partition via probability-domain scans with a FIXED per-step rescale
    (exp bias c):  a_t = exp(x_t - c) * (E^T a_{t-1}),  E = exp(trans).
  - meet-in-the-middle: the recursion is linear, so
        Z_b = a_M[b] . w_{L_b-1-M}[b]
    where w is a BACKWARD recursion w_j = E (d_{L_b-j} * w_{j-1}), w_0 = 1.
    fwd runs t=1..256 and bwd j=1..255 as two INDEPENDENT serial chains that
    pipeline on PE/DVE -- half the serial depth of a single 511-step scan.
  - the bwd exp-table is per-row time-reversed ON HOST (pure layout gather of
    logits), so the device needs no masking; rows with L_b-1 <= M instead
    capture a at t=L_b-1 (then w_cap = w_0 = ones).  Uniformly:
        logZ_b = ln(a_hist[t_a] . w_hist[j_w]) + c*L_b,
        t_a = min(L_b-1, M),  j_w = max(L_b-1-M, 0).
  - gold score: only the per-core TOTAL is needed (loss is a sum), so
      unary = one ap_gather from the transposed raw-logits tile with
              per-gpsimd-core label bucketing + masked accumulation,
      pair  = ap_gather from a replicated flat trans (mask folded into idx).
    Their reductions run on the otherwise-idle GPSIMD engine.
  - per-core partial losses summed on host.
"""

import numpy as np

B, T, K = 256, 512, 128
NCORES = 8
BL = B // NCORES          # 32 batch rows per core
M = 256                   # fwd computes a_t for t=0..M  (256 serial steps)
JMAX = 255                # bwd computes w_j for j=0..JMAX (255 serial steps)
NTF = M + 1               # fwd time slots
NTB = JMAX                # bwd j slots (j=1..JMAX stored at slot j-1)
C_LOG = 5.9               # fixed per-step log rescale (exp bias)
NIU = 1536                # padded unary slots per gpsimd core (max seen 1188)

_CACHE = {}


def _build_program():
    from contextlib import ExitStack

    import concourse.bass as bass
    import concourse.mybir as mybir
    import concourse.tile as tile
    from concourse import bacc

    f32 = mybir.dt.float32
    bf16 = mybir.dt.bfloat16
    i16 = mybir.dt.int16
    AX = mybir.AxisListType
    OP = mybir.AluOpType
    ACTF = mybir.ActivationFunctionType

    nc = bacc.Bacc("TRN2", target_bir_lowering=False, debug=False)

    CF = NTF * BL             # 8224 fwd raw/exe cols
    CB = NTB * BL             # 8160 bwd raw/exe cols
    CW = (JMAX + 1) * BL      # 8192 w_hist cols

    raw_d = nc.dram_tensor("raw_all", [128, CF + CB], bf16, kind="ExternalInput").ap()
    trans_d = nc.dram_tensor("trans", [K, K], f32, kind="ExternalInput").ap()
    transT_d = nc.dram_tensor("transT", [K, K], f32, kind="ExternalInput").ap()
    transf0_d = nc.dram_tensor("trans_f0", [K * K + 1], f32, kind="ExternalInput").ap()
    seqf_d = nc.dram_tensor("seqf_row", [1, BL], f32, kind="ExternalInput").ap()
    idxcap_d = nc.dram_tensor("idx_cap", [128, 4], i16, kind="ExternalInput").ap()
    pidx_d = nc.dram_tensor("pidx", [128, 128], i16, kind="ExternalInput").ap()
    idxu_d = nc.dram_tensor("idx_u", [128, NIU // 16], i16, kind="ExternalInput").ap()
    mual_d = nc.dram_tensor("mu_all", [128, 2 * NIU], bf16, kind="ExternalInput").ap()
    loss_d = nc.dram_tensor("loss", [1, 1], f32, kind="ExternalOutput").ap()

    with tile.TileContext(nc) as tc, ExitStack() as ctx:
        big_pool = ctx.enter_context(tc.tile_pool(name="big", bufs=1))
        small_pool = ctx.enter_context(tc.tile_pool(name="small", bufs=1))
        ps_f = ctx.enter_context(tc.tile_pool(name="psf", bufs=2, space="PSUM"))
        ps_b = ctx.enter_context(tc.tile_pool(name="psb", bufs=2, space="PSUM"))
        ps_misc = ctx.enter_context(tc.tile_pool(name="ps_misc", bufs=1, space="PSUM"))

        # ---------------- SBUF tiles ----------------
        raw_all = big_pool.tile([128, CF + CB], bf16, tag="raw_all")
        exe_f = big_pool.tile([128, CF], bf16, tag="exe_f")
        exe_b = big_pool.tile([128, CB], bf16, tag="exe_b")
        a_hist = big_pool.tile([128, CF], bf16, tag="a_hist")
        w_hist = big_pool.tile([128, CW], bf16, tag="w_hist")
        tr_rep = big_pool.tile([128, K * K + 1], f32, tag="tr_rep")

        trs = small_pool.tile([K, K], f32, tag="trs")
        trsT = small_pool.tile([K, K], f32, tag="trsT")
        e_bf = small_pool.tile([K, K], bf16, tag="e_bf")
        et_bf = small_pool.tile([K, K], bf16, tag="et_bf")
        seqf = small_pool.tile([1, BL], f32, tag="seqf")
        idx_cap = small_pool.tile([128, 4], i16, tag="idx_cap")
        pidx = small_pool.tile([128, 128], i16, tag="pidx")
        idx_u = small_pool.tile([128, NIU // 16], i16, tag="idx_u")
        mu_all = small_pool.tile([128, 2 * NIU], bf16, tag="mu_all")
        bias_c = small_pool.tile([128, 1], f32, tag="bias_c")
        ones_col = small_pool.tile([128, 1], bf16, tag="ones_col")

        gu = small_pool.tile([128, 2 * NIU], bf16, tag="gu")
        junk = small_pool.tile([128, 2 * NIU], bf16, tag="junk")
        u_acc = small_pool.tile([128, 1], f32, tag="u_acc")
        pair_g = small_pool.tile([128, 2048], f32, tag="pair_g")
        ga = small_pool.tile([128, 64], bf16, tag="ga")
        gw = small_pool.tile([128, 64], bf16, tag="gw")
        prod = small_pool.tile([128, 64], bf16, tag="prod")
        dots = small_pool.tile([1, BL], f32, tag="dots")
        ln_row = small_pool.tile([1, BL], f32, tag="ln_row")
        lc_row = small_pool.tile([1, BL], f32, tag="lc_row")
        t1 = small_pool.tile([1, 1], f32, tag="t1")
        loss_sb = small_pool.tile([1, 1], f32, tag="loss_sb")

        # ---------------- prologue ----------------
        # small inputs on the sync queue
        nc.sync.dma_start(trs[:], trans_d[:, :])
        nc.sync.dma_start(trsT[:], transT_d[:, :])
        nc.sync.dma_start(seqf[:], seqf_d[:, :])
        nc.sync.dma_start(idx_cap[:], idxcap_d[:, :])

        # raw logits: fwd part chunked on sync queue, bwd part on gpsimd queue
        FCH = [0, 1024, 3072, 5120, 7168, CF]
        BCH = [0, 1024, 3072, 5120, 7168, CB]

        def dma_f(i):
            nc.sync.dma_start(raw_all[:, FCH[i] : FCH[i + 1]], raw_d[:, FCH[i] : FCH[i + 1]])

        def dma_b(i):
            nc.gpsimd.dma_start(
                raw_all[:, CF + BCH[i] : CF + BCH[i + 1]],
                raw_d[:, CF + BCH[i] : CF + BCH[i + 1]],
            )

        dma_f(0)
        dma_b(0)

        # gather/mask tables + replicated trans on the gpsimd queue
        def emit_tables():
            nc.gpsimd.dma_start(pidx[:], pidx_d[:, :])
            nc.gpsimd.dma_start(idx_u[:], idxu_d[:, :])
            nc.gpsimd.dma_start(mu_all[:], mual_d[:, :])

        def emit_trrep(r):
            n = 2048 if r < 7 else 2049
            sl = slice(r * 2048, r * 2048 + n)
            nc.gpsimd.dma_start(
                tr_rep[:, sl], bass.AP(transf0_d.tensor, sl.start, [[0, 128], [1, n]])
            )

        nc.vector.memset(bias_c[:], -C_LOG)
        nc.vector.memset(ones_col[:], 1.0)
        nc.scalar.activation(e_bf[:], trs[:], ACTF.Exp)
        nc.scalar.activation(et_bf[:], trsT[:], ACTF.Exp)

        # exp chunks (scalar engine): 32 t-slots at a time
        def exp_f(k):
            c0, c1 = k * 1024, min((k + 1) * 1024, CF)
            nc.scalar.activation(exe_f[:, c0:c1], raw_all[:, c0:c1], ACTF.Exp, bias=bias_c[:])

        def exp_b(k):
            c0, c1 = k * 1024, min((k + 1) * 1024, CB)
            nc.scalar.activation(
                exe_b[:, c0:c1], raw_all[:, CF + c0 : CF + c1], ACTF.Exp, bias=bias_c[:]
            )

        exp_f(0)
        exp_b(0)

        # init states
        nc.vector.tensor_copy(a_hist[:, 0:BL], exe_f[:, 0:BL])
        nc.vector.memset(w_hist[:, 0:BL], 1.0)

        # ---------------- the two scans, interleaved ----------------
        for s in range(1, M + 1):
            if s in (2, 18, 34, 50):
                i = (s - 2) // 16 + 1
                dma_f(i)
                dma_b(i)
            if s == 66:
                emit_tables()
            if 70 <= s < 102 and (s - 70) % 4 == 0:
                emit_trrep((s - 70) // 4)
            if s % 32 == 8:
                k = s // 32 + 1
                if k * 1024 < CF:
                    exp_f(k)
            if s % 32 == 24:
                k = s // 32 + 1
                if k * 1024 < CB:
                    exp_b(k)

            # fwd step t=s:  a_s = exe_f[s] * (E^T a_{s-1})
            up_f = ps_f.tile([K, BL], f32, tag="up_f")
            nc.tensor.matmul(
                up_f[:], e_bf[:], a_hist[:, (s - 1) * BL : s * BL], start=True, stop=True
            )
            nc.vector.tensor_mul(
                a_hist[:, s * BL : (s + 1) * BL], up_f[:], exe_f[:, s * BL : (s + 1) * BL]
            )

            # bwd step j=s:  w_s = exe_b[s-1] * (E w_{s-1})
            if s <= JMAX:
                up_b = ps_b.tile([K, BL], f32, tag="up_b")
                nc.tensor.matmul(
                    up_b[:], et_bf[:], w_hist[:, (s - 1) * BL : s * BL], start=True, stop=True
                )
                nc.vector.tensor_mul(
                    w_hist[:, s * BL : (s + 1) * BL], up_b[:], exe_b[:, (s - 1) * BL : s * BL]
                )

        # ---------------- gold score (gathers run during the scan) ----------------
        # unary: one bucketed gather from raw_all + masked accumulation
        nc.gpsimd.ap_gather(
            gu[:], raw_all[:], idx_u[:, :], channels=128,
            num_elems=(CF + CB) // 2, d=2, num_idxs=NIU,
        )
        nc.vector.scalar_tensor_tensor(
            junk[:], gu[:], 1.0, mu_all[:], OP.mult, OP.mult,
            accum_out=u_acc[:, 0:1],
        )
        # pair: gather from replicated flat trans (idx 16384 -> 0.0 pad);
        # each gpsimd core's gather is replicated over its 16 partitions, so
        # the sum counts every pair 16x -- folded into the 1/16 below.
        nc.gpsimd.ap_gather(
            pair_g[:], tr_rep[:], pidx[:, :], channels=128,
            num_elems=K * K + 1, d=1, num_idxs=2048,
        )
        pair_acc = small_pool.tile([128, 1], f32, tag="pair_acc")
        pair_junk = small_pool.tile([128, 2048], f32, tag="pair_junk")
        nc.scalar.activation(
            pair_junk[:], pair_g[:], ACTF.Copy, accum_out=pair_acc[:]
        )
        # score_tot = sum_p(u_acc) + sum_p(pair_acc)/16 via PE column sums
        ones_f = small_pool.tile([128, 1], f32, tag="ones_f")
        nc.vector.memset(ones_f[:], 1.0)
        c116_f = small_pool.tile([128, 1], f32, tag="c116_f")
        nc.vector.memset(c116_f[:], 1.0 / 16.0)
        sc_ps = ps_misc.tile([1, 1], f32, tag="mm_fin")
        nc.tensor.matmul(sc_ps[:], ones_f[:], u_acc[:], start=True, stop=False)
        nc.tensor.matmul(sc_ps[:], c116_f[:], pair_acc[:], start=False, stop=True)
        score_tot = small_pool.tile([1, 1], f32, tag="score_tot")
        nc.vector.tensor_copy(score_tot[:], sc_ps[:])

        # ---------------- capture + logZ + loss ----------------
        nc.gpsimd.ap_gather(
            ga[:], a_hist[:], idx_cap[:, 0:2], channels=128,
            num_elems=CF // 2, d=2, num_idxs=32,
        )
        nc.gpsimd.ap_gather(
            gw[:], w_hist[:], idx_cap[:, 2:4], channels=128,
            num_elems=CW // 2, d=2, num_idxs=32,
        )
        nc.vector.tensor_mul(prod[:], ga[:], gw[:])
        dots_ev = ps_misc.tile([1, 16], f32, tag="mm_ev")
        nc.tensor.matmul(dots_ev[:], ones_col[:], prod[:, 0:64:4], start=True, stop=True)
        dots_od = ps_misc.tile([1, 16], f32, tag="mm_od")
        nc.tensor.matmul(dots_od[:], ones_col[:], prod[:, 3:64:4], start=True, stop=True)
        nc.vector.tensor_copy(dots[:, 0:BL:2], dots_ev[:])
        nc.vector.tensor_copy(dots[:, 1:BL:2], dots_od[:])
        nc.scalar.activation(ln_row[:], dots[:], ACTF.Ln)
        # lc = ln(dot) + c*L
        nc.vector.scalar_tensor_tensor(
            lc_row[:], seqf[:], C_LOG, ln_row[:], OP.mult, OP.add
        )
        nc.vector.tensor_reduce(t1[:], lc_row[:], AX.X, OP.add)
        nc.vector.tensor_sub(loss_sb[:], t1[:], score_tot[:])
        nc.sync.dma_start(loss_d[:, :], loss_sb[:])

    nc.compile()
    return nc


def _get_program():
    if "prog" not in _CACHE:
        _CACHE["prog"] = _build_program()
    return _CACHE["prog"]


def _core_tables(lgT_bf, lab, L):
    """Per-core tables: raw_all layout + gather indices/masks.

    lgT_bf: [K, T, BL] bf16 transposed logits, lab: [BL, T] int32, L: [BL]."""
    import ml_dtypes

    bf = ml_dtypes.bfloat16
    t = {}
    # raw_all: fwd t=0..M, then bwd j=1..JMAX time-reversed per row
    raw_f = lgT_bf[:, : M + 1, :].reshape(128, -1)
    tidx = np.maximum(L[None, :] - np.arange(1, JMAX + 1)[:, None], 0)  # [j, b]
    raw_b = lgT_bf[:, tidx, np.arange(BL)[None, :]].reshape(128, -1)
    t["raw_all"] = np.ascontiguousarray(
        np.concatenate([raw_f, raw_b], axis=1), dtype=bf
    )

    # capture indices (d=2 units): slot i=b lives at idx-col (c= b//16, pp=b%16)
    p = np.arange(128)[:, None]
    cgrid = np.arange(2)[None, :]
    bcap = cgrid * 16 + (p % 16)
    ta = np.minimum(L - 1, M)
    jw = np.maximum(L - 1 - M, 0)
    idx_a = (ta[bcap] * 16 + bcap // 2).astype(np.int16)
    idx_w = (jw[bcap] * 16 + bcap // 2).astype(np.int16)
    t["idx_cap"] = np.concatenate([idx_a, idx_w], axis=1)

    # pair idx: gpsimd core g handles rows 4g..4g+3, slot s=(col*16+pp) -> (r,tt)
    lab_n = np.concatenate([lab[:, 1:], np.zeros((BL, 1), np.int64)], axis=1)
    pid = lab.astype(np.int64) * 128 + lab_n  # value for pair (t, t+1)
    act = (np.arange(T)[None, :] + 1) < L[:, None]  # t+1 <= L-1
    pidv = np.where(act, pid, 16384)  # [BL, T]; slot t=511 always padded
    pidx = np.zeros((128, 128), np.int32)
    for g in range(8):
        rows = pidv[4 * g : 4 * g + 4].reshape(-1)  # [2048] slots r*512+tt
        s = np.arange(2048)
        pidx[16 * g + (s % 16), s // 16] = rows
    t["pidx"] = pidx.astype(np.int16)

    # unary: bucket active (b,t) entries by label's gpsimd core
    bb, tt = np.nonzero(np.arange(T)[None, :] < L[:, None])
    kk = lab[bb, tt]
    fwd_side = tt <= M
    unit = np.where(
        fwd_side, tt * 16 + bb // 2, (M + (L[bb] - tt)) * 16 + bb // 2
    ).astype(np.int64)
    par = (bb & 1).astype(np.int64)
    core = kk >> 4
    owner = kk & 15
    order = np.argsort(core, kind="stable")
    core_s, unit_s, owner_s, par_s = core[order], unit[order], owner[order], par[order]
    counts = np.bincount(core_s, minlength=8)
    assert counts.max() <= NIU, f"unary bucket overflow: {counts.max()}"
    idx_flat = np.zeros((8, NIU), np.int64)
    own_flat = np.full((8, NIU), -1, np.int64)
    par_flat = np.zeros((8, NIU), np.int64)
    off = 0
    for g in range(8):
        n = counts[g]
        idx_flat[g, :n] = unit_s[off : off + n]
        own_flat[g, :n] = owner_s[off : off + n]
        par_flat[g, :n] = par_s[off : off + n]
        off += n
    idx_u = np.zeros((128, NIU // 16), np.int16)
    s = np.arange(NIU)
    for g in range(8):
        idx_u[16 * g + (s % 16), s // 16] = idx_flat[g].astype(np.int16)
    t["idx_u"] = idx_u
    pp16 = np.arange(16)
    mu_all = np.zeros((128, 2 * NIU), np.float32)
    for g in range(8):
        own_match = own_flat[g][None, :] == pp16[:, None]  # [16, NIU]
        mu_all[16 * g : 16 * g + 16, 0::2] = own_match & (par_flat[g][None, :] == 0)
        mu_all[16 * g : 16 * g + 16, 1::2] = own_match & (par_flat[g][None, :] == 1)
    t["mu_all"] = mu_all.astype(bf)
    return t


def _make_in_maps(logits, labels, seq_lens, trans):
    import ml_dtypes

    bf = ml_dtypes.bfloat16
    logits = np.asarray(logits, dtype=np.float32)
    labels = np.asarray(labels, dtype=np.int64)
    seq_lens = np.asarray(seq_lens, dtype=np.int64)
    trans = np.asarray(trans, dtype=np.float32)
    trans_f0 = np.append(trans.reshape(-1), np.float32(0)).astype(np.float32)
    transT = np.ascontiguousarray(trans.T)

    in_maps = []
    for c in range(NCORES):
        sl = slice(c * BL, (c + 1) * BL)
        lgT_bf = logits[sl].transpose(2, 1, 0).astype(bf)  # [K, T, BL]
        L = seq_lens[sl]
        m = {
            "trans": trans,
            "transT": transT,
            "trans_f0": trans_f0,
            "seqf_row": L.astype(np.float32).reshape(1, BL),
        }
        m.update(_core_tables(lgT_bf, labels[sl], L))
        in_maps.append(m)
    return in_maps


def kernel(logits, labels, seq_lens, trans):
    from concourse.bass_utils import run_bass_kernel_spmd

    nc = _get_program()
    in_maps = _make_in_maps(logits, labels, seq_lens, trans)
    res = run_bass_kernel_spmd(nc, in_maps, list(range(NCORES)))
    total = sum(float(res.results[c]["loss"][0, 0]) for c in range(NCORES))
    return np.float32(total)


# revision 31
# speedup vs baseline: 1.5999x; 1.0924x over previous
"""CRF negative-log-likelihood loss kernel for Trainium2 (Bass/Tile).

Strategy (data-parallel over batch, 8 NeuronCores, 32 rows each):
  - log-partition via probability-domain scans with a FIXED per-step rescale
    (exp bias c):  a_t = exp(x_t - c) * (E^T a_{t-1}),  E = exp(trans).
  - meet-in-the-middle: the recursion is linear, so
        Z_b = a_M[b] . w_{L_b-1-M}[b]
    where w is a BACKWARD recursion w_j = E (d_{L_b-j} * w_{j-1}), w_0 = 1.
    fwd runs t=1..256 and bwd j=1..255 as two INDEPENDENT serial chains that
    pipeline on PE/DVE -- half the serial depth of a single 511-step scan.
  - the bwd exp-table is per-row time-reversed ON HOST (pure layout gather of
    logits), so the device needs no masking; rows with L_b-1 <= M instead
    capture a at t=L_b-1 (then w_cap = w_0 = ones).  Uniformly:
        logZ_b = ln(a_hist[t_a] . w_hist[j_w]) + c*L_b,
        t_a = min(L_b-1, M),  j_w = max(L_b-1-M, 0).
  - gold score: only the per-core TOTAL is needed (loss is a sum), so
      unary = one ap_gather from the transposed raw-logits tile with
              per-gpsimd-core label bucketing + masked accumulation,
      pair  = ap_gather from a replicated flat trans (mask folded into idx).
    Their reductions run on the otherwise-idle GPSIMD engine.
  - per-core partial losses summed on host.
"""

import numpy as np

B, T, K = 256, 512, 128
NCORES = 8
BL = B // NCORES          # 32 batch rows per core
M = 256                   # fwd computes a_t for t=0..M  (256 serial steps)
JMAX = 255                # bwd computes w_j for j=0..JMAX (255 serial steps)
NTF = M + 1               # fwd time slots
NTB = JMAX                # bwd j slots (j=1..JMAX stored at slot j-1)
C_LOG = 5.9               # fixed per-step log rescale (exp bias)
NIU = 1536                # padded unary slots per gpsimd core (max seen 1188)

_CACHE = {}


def _build_program():
    from contextlib import ExitStack

    import concourse.bass as bass
    import concourse.mybir as mybir
    import concourse.tile as tile
    from concourse import bacc

    f32 = mybir.dt.float32
    bf16 = mybir.dt.bfloat16
    i16 = mybir.dt.int16
    AX = mybir.AxisListType
    OP = mybir.AluOpType
    ACTF = mybir.ActivationFunctionType

    nc = bacc.Bacc("TRN2", target_bir_lowering=False, debug=False)

    CF = NTF * BL             # 8224 fwd raw/exe cols
    CB = NTB * BL             # 8160 bwd raw/exe cols
    CW = (JMAX + 1) * BL      # 8192 w_hist cols

    raw_d = nc.dram_tensor("raw_all", [128, CF + CB], bf16, kind="ExternalInput").ap()
    trans_d = nc.dram_tensor("trans", [K, K], f32, kind="ExternalInput").ap()
    transT_d = nc.dram_tensor("transT", [K, K], f32, kind="ExternalInput").ap()
    cmat_d = nc.dram_tensor("cmat", [K, K], f32, kind="ExternalInput").ap()
    seqf_d = nc.dram_tensor("seqf_row", [1, BL], f32, kind="ExternalInput").ap()
    idxcap_d = nc.dram_tensor("idx_cap", [128, 4], i16, kind="ExternalInput").ap()
    idxu_d = nc.dram_tensor("idx_u", [128, NIU // 16], i16, kind="ExternalInput").ap()
    mual_d = nc.dram_tensor("mu_all", [128, 2 * NIU], bf16, kind="ExternalInput").ap()
    loss_d = nc.dram_tensor("loss", [1, 1], f32, kind="ExternalOutput").ap()

    with tile.TileContext(nc) as tc, ExitStack() as ctx:
        big_pool = ctx.enter_context(tc.tile_pool(name="big", bufs=1))
        small_pool = ctx.enter_context(tc.tile_pool(name="small", bufs=1))
        ps_f = ctx.enter_context(tc.tile_pool(name="psf", bufs=2, space="PSUM"))
        ps_b = ctx.enter_context(tc.tile_pool(name="psb", bufs=2, space="PSUM"))
        ps_misc = ctx.enter_context(tc.tile_pool(name="ps_misc", bufs=1, space="PSUM"))

        # ---------------- SBUF tiles ----------------
        raw_all = big_pool.tile([128, CF + CB], bf16, tag="raw_all")
        exe_f = big_pool.tile([128, CF], bf16, tag="exe_f")
        exe_b = big_pool.tile([128, CB], bf16, tag="exe_b")
        a_hist = big_pool.tile([128, CF], bf16, tag="a_hist")
        w_hist = big_pool.tile([128, CW], bf16, tag="w_hist")

        trs = small_pool.tile([K, K], f32, tag="trs")
        trsT = small_pool.tile([K, K], f32, tag="trsT")
        cmat = small_pool.tile([K, K], f32, tag="cmat")
        e_bf = small_pool.tile([K, K], bf16, tag="e_bf")
        et_bf = small_pool.tile([K, K], bf16, tag="et_bf")
        seqf = small_pool.tile([1, BL], f32, tag="seqf")
        idx_cap = small_pool.tile([128, 4], i16, tag="idx_cap")
        idx_u = small_pool.tile([128, NIU // 16], i16, tag="idx_u")
        mu_all = small_pool.tile([128, 2 * NIU], bf16, tag="mu_all")
        bias_c = small_pool.tile([128, 1], f32, tag="bias_c")
        ones_col = small_pool.tile([128, 1], bf16, tag="ones_col")

        gu = small_pool.tile([128, 2 * NIU], bf16, tag="gu")
        junk = small_pool.tile([128, 2 * NIU], bf16, tag="junk")
        u_acc = small_pool.tile([128, 1], f32, tag="u_acc")
        ga = small_pool.tile([128, 64], bf16, tag="ga")
        gw = small_pool.tile([128, 64], bf16, tag="gw")
        prod = small_pool.tile([128, 64], bf16, tag="prod")
        dots = small_pool.tile([1, BL], f32, tag="dots")
        ln_row = small_pool.tile([1, BL], f32, tag="ln_row")
        lc_row = small_pool.tile([1, BL], f32, tag="lc_row")
        t1 = small_pool.tile([1, 1], f32, tag="t1")
        loss_sb = small_pool.tile([1, 1], f32, tag="loss_sb")

        # ---------------- prologue ----------------
        # small inputs on the sync queue
        nc.sync.dma_start(trs[:], trans_d[:, :])
        nc.sync.dma_start(trsT[:], transT_d[:, :])
        nc.sync.dma_start(seqf[:], seqf_d[:, :])
        nc.sync.dma_start(idx_cap[:], idxcap_d[:, :])

        # raw logits: fwd part chunked on sync queue, bwd part on gpsimd queue
        FCH = [0, 1024, 3072, 5120, 7168, CF]
        BCH = [0, 1024, 3072, 5120, 7168, CB]

        def dma_f(i):
            nc.sync.dma_start(raw_all[:, FCH[i] : FCH[i + 1]], raw_d[:, FCH[i] : FCH[i + 1]])

        def dma_b(i):
            nc.gpsimd.dma_start(
                raw_all[:, CF + BCH[i] : CF + BCH[i + 1]],
                raw_d[:, CF + BCH[i] : CF + BCH[i + 1]],
            )

        dma_f(0)
        dma_b(0)

        # gather/mask tables on the gpsimd queue
        def emit_tables():
            nc.gpsimd.dma_start(cmat[:], cmat_d[:, :])
            nc.gpsimd.dma_start(idx_u[:], idxu_d[:, :])
            nc.gpsimd.dma_start(mu_all[:], mual_d[:, :])

        nc.vector.memset(bias_c[:], -C_LOG)
        nc.vector.memset(ones_col[:], 1.0)
        nc.scalar.activation(e_bf[:], trs[:], ACTF.Exp)
        nc.scalar.activation(et_bf[:], trsT[:], ACTF.Exp)

        # exp chunks (scalar engine): 32 t-slots at a time
        def exp_f(k):
            c0, c1 = k * 1024, min((k + 1) * 1024, CF)
            nc.scalar.activation(exe_f[:, c0:c1], raw_all[:, c0:c1], ACTF.Exp, bias=bias_c[:])

        def exp_b(k):
            c0, c1 = k * 1024, min((k + 1) * 1024, CB)
            nc.scalar.activation(
                exe_b[:, c0:c1], raw_all[:, CF + c0 : CF + c1], ACTF.Exp, bias=bias_c[:]
            )

        exp_f(0)
        exp_b(0)

        # init states
        nc.vector.tensor_copy(a_hist[:, 0:BL], exe_f[:, 0:BL])
        nc.vector.memset(w_hist[:, 0:BL], 1.0)

        # ---------------- the two scans, interleaved ----------------
        for s in range(1, M + 1):
            if s in (2, 18, 34, 50):
                i = (s - 2) // 16 + 1
                dma_f(i)
                dma_b(i)
            if s == 66:
                emit_tables()
            if s % 32 == 8:
                k = s // 32 + 1
                if k * 1024 < CF:
                    exp_f(k)
            if s % 32 == 24:
                k = s // 32 + 1
                if k * 1024 < CB:
                    exp_b(k)

            # fwd step t=s:  a_s = exe_f[s] * (E^T a_{s-1})
            up_f = ps_f.tile([K, BL], f32, tag="up_f")
            nc.tensor.matmul(
                up_f[:], e_bf[:], a_hist[:, (s - 1) * BL : s * BL], start=True, stop=True
            )
            nc.vector.tensor_mul(
                a_hist[:, s * BL : (s + 1) * BL], up_f[:], exe_f[:, s * BL : (s + 1) * BL]
            )

            # bwd step j=s:  w_s = exe_b[s-1] * (E w_{s-1})
            if s <= JMAX:
                up_b = ps_b.tile([K, BL], f32, tag="up_b")
                nc.tensor.matmul(
                    up_b[:], et_bf[:], w_hist[:, (s - 1) * BL : s * BL], start=True, stop=True
                )
                nc.vector.tensor_mul(
                    w_hist[:, s * BL : (s + 1) * BL], up_b[:], exe_b[:, (s - 1) * BL : s * BL]
                )

        # ---------------- gold score (gathers run during the scan) ----------------
        # unary: one bucketed gather from raw_all + masked accumulation
        nc.gpsimd.ap_gather(
            gu[:], raw_all[:], idx_u[:, :], channels=128,
            num_elems=(CF + CB) // 2, d=2, num_idxs=NIU,
        )
        nc.vector.scalar_tensor_tensor(
            junk[:], gu[:], 1.0, mu_all[:], OP.mult, OP.mult,
            accum_out=u_acc[:, 0:1],
        )
        # pair: trans contracted against the host-computed transition-count
        # matrix C (labels and mask are host-known): pair_tot = <C, trans>.
        pair_acc = small_pool.tile([128, 1], f32, tag="pair_acc")
        pair_junk = small_pool.tile([128, K], f32, tag="pair_junk")
        nc.vector.scalar_tensor_tensor(
            pair_junk[:], cmat[:], 1.0, trs[:], OP.mult, OP.mult,
            accum_out=pair_acc[:],
        )
        # score_tot = sum_p(u_acc + pair_acc) via PE column sum
        ones_f = small_pool.tile([128, 1], f32, tag="ones_f")
        nc.vector.memset(ones_f[:], 1.0)
        sc_ps = ps_misc.tile([1, 1], f32, tag="mm_fin")
        nc.tensor.matmul(sc_ps[:], ones_f[:], u_acc[:], start=True, stop=False)
        nc.tensor.matmul(sc_ps[:], ones_f[:], pair_acc[:], start=False, stop=True)
        score_tot = small_pool.tile([1, 1], f32, tag="score_tot")
        nc.vector.tensor_copy(score_tot[:], sc_ps[:])

        # ---------------- capture + logZ + loss ----------------
        nc.gpsimd.ap_gather(
            ga[:], a_hist[:], idx_cap[:, 0:2], channels=128,
            num_elems=CF // 2, d=2, num_idxs=32,
        )
        nc.gpsimd.ap_gather(
            gw[:], w_hist[:], idx_cap[:, 2:4], channels=128,
            num_elems=CW // 2, d=2, num_idxs=32,
        )
        nc.vector.tensor_mul(prod[:], ga[:], gw[:])
        dots_ev = ps_misc.tile([1, 16], f32, tag="mm_ev")
        nc.tensor.matmul(dots_ev[:], ones_col[:], prod[:, 0:64:4], start=True, stop=True)
        dots_od = ps_misc.tile([1, 16], f32, tag="mm_od")
        nc.tensor.matmul(dots_od[:], ones_col[:], prod[:, 3:64:4], start=True, stop=True)
        nc.vector.tensor_copy(dots[:, 0:BL:2], dots_ev[:])
        nc.vector.tensor_copy(dots[:, 1:BL:2], dots_od[:])
        nc.scalar.activation(ln_row[:], dots[:], ACTF.Ln)
        # lc = ln(dot) + c*L
        nc.vector.scalar_tensor_tensor(
            lc_row[:], seqf[:], C_LOG, ln_row[:], OP.mult, OP.add
        )
        nc.vector.tensor_reduce(t1[:], lc_row[:], AX.X, OP.add)
        nc.vector.tensor_sub(loss_sb[:], t1[:], score_tot[:])
        nc.sync.dma_start(loss_d[:, :], loss_sb[:])

    nc.compile()
    return nc


def _get_program():
    if "prog" not in _CACHE:
        _CACHE["prog"] = _build_program()
    return _CACHE["prog"]


def _core_tables(lgT_bf, lab, L):
    """Per-core tables: raw_all layout + gather indices/masks.

    lgT_bf: [K, T, BL] bf16 transposed logits, lab: [BL, T] int32, L: [BL]."""
    import ml_dtypes

    bf = ml_dtypes.bfloat16
    t = {}
    # raw_all: fwd t=0..M, then bwd j=1..JMAX time-reversed per row
    raw_f = lgT_bf[:, : M + 1, :].reshape(128, -1)
    tidx = np.maximum(L[None, :] - np.arange(1, JMAX + 1)[:, None], 0)  # [j, b]
    raw_b = lgT_bf[:, tidx, np.arange(BL)[None, :]].reshape(128, -1)
    t["raw_all"] = np.ascontiguousarray(
        np.concatenate([raw_f, raw_b], axis=1), dtype=bf
    )

    # capture indices (d=2 units): slot i=b lives at idx-col (c= b//16, pp=b%16)
    p = np.arange(128)[:, None]
    cgrid = np.arange(2)[None, :]
    bcap = cgrid * 16 + (p % 16)
    ta = np.minimum(L - 1, M)
    jw = np.maximum(L - 1 - M, 0)
    idx_a = (ta[bcap] * 16 + bcap // 2).astype(np.int16)
    idx_w = (jw[bcap] * 16 + bcap // 2).astype(np.int16)
    t["idx_cap"] = np.concatenate([idx_a, idx_w], axis=1)

    # pair: transition-count matrix C[i,j] = #{(b,t): lab=i->j, t+1 < L_b}
    act = (np.arange(T - 1)[None, :] + 1) < L[:, None]
    i_lab = lab[:, :-1][act]
    j_lab = lab[:, 1:][act]
    cmat = np.zeros((K, K), np.float32)
    np.add.at(cmat, (i_lab, j_lab), 1.0)
    t["cmat"] = cmat

    # unary: bucket active (b,t) entries by label's gpsimd core
    bb, tt = np.nonzero(np.arange(T)[None, :] < L[:, None])
    kk = lab[bb, tt]
    fwd_side = tt <= M
    unit = np.where(
        fwd_side, tt * 16 + bb // 2, (M + (L[bb] - tt)) * 16 + bb // 2
    ).astype(np.int64)
    par = (bb & 1).astype(np.int64)
    core = kk >> 4
    owner = kk & 15
    order = np.argsort(core, kind="stable")
    core_s, unit_s, owner_s, par_s = core[order], unit[order], owner[order], par[order]
    counts = np.bincount(core_s, minlength=8)
    assert counts.max() <= NIU, f"unary bucket overflow: {counts.max()}"
    idx_flat = np.zeros((8, NIU), np.int64)
    own_flat = np.full((8, NIU), -1, np.int64)
    par_flat = np.zeros((8, NIU), np.int64)
    off = 0
    for g in range(8):
        n = counts[g]
        idx_flat[g, :n] = unit_s[off : off + n]
        own_flat[g, :n] = owner_s[off : off + n]
        par_flat[g, :n] = par_s[off : off + n]
        off += n
    idx_u = np.zeros((128, NIU // 16), np.int16)
    s = np.arange(NIU)
    for g in range(8):
        idx_u[16 * g + (s % 16), s // 16] = idx_flat[g].astype(np.int16)
    t["idx_u"] = idx_u
    pp16 = np.arange(16)
    mu_all = np.zeros((128, 2 * NIU), np.float32)
    for g in range(8):
        own_match = own_flat[g][None, :] == pp16[:, None]  # [16, NIU]
        mu_all[16 * g : 16 * g + 16, 0::2] = own_match & (par_flat[g][None, :] == 0)
        mu_all[16 * g : 16 * g + 16, 1::2] = own_match & (par_flat[g][None, :] == 1)
    t["mu_all"] = mu_all.astype(bf)
    return t


def _make_in_maps(logits, labels, seq_lens, trans):
    import ml_dtypes

    bf = ml_dtypes.bfloat16
    logits = np.asarray(logits, dtype=np.float32)
    labels = np.asarray(labels, dtype=np.int64)
    seq_lens = np.asarray(seq_lens, dtype=np.int64)
    trans = np.asarray(trans, dtype=np.float32)
    transT = np.ascontiguousarray(trans.T)

    in_maps = []
    for c in range(NCORES):
        sl = slice(c * BL, (c + 1) * BL)
        lgT_bf = logits[sl].transpose(2, 1, 0).astype(bf)  # [K, T, BL]
        L = seq_lens[sl]
        m = {
            "trans": trans,
            "transT": transT,
            "seqf_row": L.astype(np.float32).reshape(1, BL),
        }
        m.update(_core_tables(lgT_bf, labels[sl], L))
        in_maps.append(m)
    return in_maps


def kernel(logits, labels, seq_lens, trans):
    from concourse.bass_utils import run_bass_kernel_spmd

    nc = _get_program()
    in_maps = _make_in_maps(logits, labels, seq_lens, trans)
    res = run_bass_kernel_spmd(nc, in_maps, list(range(NCORES)))
    total = sum(float(res.results[c]["loss"][0, 0]) for c in range(NCORES))
    return np.float32(total)


# revision 33
# speedup vs baseline: 1.7288x; 1.0806x over previous
"""CRF negative-log-likelihood loss kernel for Trainium2 (Bass/Tile).

Strategy (data-parallel over batch, 8 NeuronCores, 32 rows each):
  - log-partition via probability-domain scans with a FIXED per-step rescale
    (exp bias c):  a_t = exp(x_t - c) * (E^T a_{t-1}),  E = exp(trans).
  - meet-in-the-middle: the recursion is linear, so
        Z_b = a_M[b] . w_{L_b-1-M}[b]
    where w is a BACKWARD recursion w_j = E (d_{L_b-j} * w_{j-1}), w_0 = 1.
    fwd runs t=1..256 and bwd j=1..255 as two INDEPENDENT serial chains that
    pipeline on PE/DVE -- half the serial depth of a single 511-step scan.
  - the bwd exp-table is per-row time-reversed ON HOST (pure layout gather of
    logits), so the device needs no masking; rows with L_b-1 <= M instead
    capture a at t=L_b-1 (then w_cap = w_0 = ones).  Uniformly:
        logZ_b = ln(a_hist[t_a] . w_hist[j_w]) + c*L_b,
        t_a = min(L_b-1, M),  j_w = max(L_b-1-M, 0).
  - gold score: only the per-core TOTAL is needed (loss is a sum), so
      unary = one ap_gather from the transposed raw-logits tile with
              per-gpsimd-core label bucketing + masked accumulation,
      pair  = ap_gather from a replicated flat trans (mask folded into idx).
    Their reductions run on the otherwise-idle GPSIMD engine.
  - per-core partial losses summed on host.
"""

import numpy as np

B, T, K = 256, 512, 128
NCORES = 8
BL = B // NCORES          # 32 batch rows per core
M = 256                   # fwd computes a_t for t=0..M  (256 serial steps)
JMAX = 255                # bwd computes w_j for j=0..JMAX (255 serial steps)
NTF = M + 1               # fwd time slots
NTB = JMAX                # bwd j slots (j=1..JMAX stored at slot j-1)
C_LOG = 5.9               # fixed per-step log rescale (exp bias)
NIU = 1536                # padded unary slots per gpsimd core (max seen 1188)

_CACHE = {}


def _build_program():
    from contextlib import ExitStack

    import concourse.bass as bass
    import concourse.mybir as mybir
    import concourse.tile as tile
    from concourse import bacc

    f32 = mybir.dt.float32
    bf16 = mybir.dt.bfloat16
    i16 = mybir.dt.int16
    AX = mybir.AxisListType
    OP = mybir.AluOpType
    ACTF = mybir.ActivationFunctionType

    nc = bacc.Bacc("TRN2", target_bir_lowering=False, debug=False)

    CF = NTF * BL             # 8224 fwd raw/exe cols
    CB = NTB * BL             # 8160 bwd raw/exe cols
    CW = (JMAX + 1) * BL      # 8192 w_hist cols

    raw_d = nc.dram_tensor("raw_all", [128, CF + CB], bf16, kind="ExternalInput").ap()
    trans_d = nc.dram_tensor("trans", [K, K], f32, kind="ExternalInput").ap()
    transT_d = nc.dram_tensor("transT", [K, K], f32, kind="ExternalInput").ap()
    cmat_d = nc.dram_tensor("cmat", [K, K], f32, kind="ExternalInput").ap()
    seqf_d = nc.dram_tensor("seqf_row", [1, BL], f32, kind="ExternalInput").ap()
    idxcap_d = nc.dram_tensor("idx_cap", [128, 4], i16, kind="ExternalInput").ap()
    idxu_d = nc.dram_tensor("idx_u", [128, NIU // 16], i16, kind="ExternalInput").ap()
    mual_d = nc.dram_tensor("mu_all", [128, 2 * NIU], bf16, kind="ExternalInput").ap()
    loss_d = nc.dram_tensor("loss", [1, 1], f32, kind="ExternalOutput").ap()

    with tile.TileContext(nc) as tc, ExitStack() as ctx:
        big_pool = ctx.enter_context(tc.tile_pool(name="big", bufs=1))
        small_pool = ctx.enter_context(tc.tile_pool(name="small", bufs=1))
        ps_f = ctx.enter_context(tc.tile_pool(name="psf", bufs=2, space="PSUM"))
        ps_b = ctx.enter_context(tc.tile_pool(name="psb", bufs=2, space="PSUM"))
        ps_misc = ctx.enter_context(tc.tile_pool(name="ps_misc", bufs=1, space="PSUM"))

        # ---------------- SBUF tiles ----------------
        raw_all = big_pool.tile([128, CF + CB], bf16, tag="raw_all")
        exe_f = big_pool.tile([128, CF], bf16, tag="exe_f")
        exe_b = big_pool.tile([128, CB], bf16, tag="exe_b")
        a_hist = big_pool.tile([128, CF], bf16, tag="a_hist")
        w_hist = big_pool.tile([128, CW], bf16, tag="w_hist")

        trs = small_pool.tile([K, K], f32, tag="trs")
        trsT = small_pool.tile([K, K], f32, tag="trsT")
        cmat = small_pool.tile([K, K], f32, tag="cmat")
        e_bf = small_pool.tile([K, K], bf16, tag="e_bf")
        et_bf = small_pool.tile([K, K], bf16, tag="et_bf")
        seqf = small_pool.tile([1, BL], f32, tag="seqf")
        idx_cap = small_pool.tile([128, 4], i16, tag="idx_cap")
        idx_u = small_pool.tile([128, NIU // 16], i16, tag="idx_u")
        mu_all = small_pool.tile([128, 2 * NIU], bf16, tag="mu_all")
        bias_c = small_pool.tile([128, 1], f32, tag="bias_c")
        ones_col = small_pool.tile([128, 1], bf16, tag="ones_col")

        gu = small_pool.tile([128, 2 * NIU], bf16, tag="gu")
        junk = small_pool.tile([128, 2 * NIU], bf16, tag="junk")
        u_acc = small_pool.tile([128, 1], f32, tag="u_acc")
        ga = small_pool.tile([128, 64], bf16, tag="ga")
        gw = small_pool.tile([128, 64], bf16, tag="gw")
        prod = small_pool.tile([128, 64], bf16, tag="prod")
        dots = small_pool.tile([1, BL], f32, tag="dots")
        ln_row = small_pool.tile([1, BL], f32, tag="ln_row")
        lc_row = small_pool.tile([1, BL], f32, tag="lc_row")
        t1 = small_pool.tile([1, 1], f32, tag="t1")
        loss_sb = small_pool.tile([1, 1], f32, tag="loss_sb")

        # ---------------- prologue ----------------
        # small inputs on the sync queue
        nc.sync.dma_start(trs[:], trans_d[:, :])
        nc.sync.dma_start(trsT[:], transT_d[:, :])
        nc.sync.dma_start(seqf[:], seqf_d[:, :])
        nc.sync.dma_start(idx_cap[:], idxcap_d[:, :])

        # raw logits: fwd part chunked on sync queue, bwd part on gpsimd queue
        FCH = [0, 1024, 3072, 5120, 7168, CF]
        BCH = [0, 1024, 3072, 5120, 7168, CB]

        def dma_f(i):
            nc.sync.dma_start(raw_all[:, FCH[i] : FCH[i + 1]], raw_d[:, FCH[i] : FCH[i + 1]])

        def dma_b(i):
            nc.sync.dma_start(
                raw_all[:, CF + BCH[i] : CF + BCH[i + 1]],
                raw_d[:, CF + BCH[i] : CF + BCH[i + 1]],
            )

        # trigger the gpsimd custom-op library load NOW (takes ~43us in the
        # background); keeps the real gathers from stalling mid-scan.
        dum_src = small_pool.tile([128, 4], bf16, tag="dum_src")
        dum_idx = small_pool.tile([128, 1], i16, tag="dum_idx")
        dum_out = small_pool.tile([128, 32], bf16, tag="dum_out")
        nc.gpsimd.memset(dum_src[:], 0.0)
        nc.gpsimd.memset(dum_idx[:], 0)
        nc.gpsimd.ap_gather(
            dum_out[:], dum_src[:], dum_idx[:], channels=128,
            num_elems=2, d=2, num_idxs=16,
        )

        dma_f(0)
        dma_b(0)

        # gather/mask tables on the sync queue
        def emit_tables():
            nc.sync.dma_start(cmat[:], cmat_d[:, :])
            nc.sync.dma_start(idx_u[:], idxu_d[:, :])
            nc.sync.dma_start(mu_all[:], mual_d[:, :])

        nc.vector.memset(bias_c[:], -C_LOG)
        nc.vector.memset(ones_col[:], 1.0)
        nc.scalar.activation(e_bf[:], trs[:], ACTF.Exp)
        nc.scalar.activation(et_bf[:], trsT[:], ACTF.Exp)

        # exp chunks (scalar engine): 32 t-slots at a time
        def exp_f(k):
            c0, c1 = k * 1024, min((k + 1) * 1024, CF)
            nc.scalar.activation(exe_f[:, c0:c1], raw_all[:, c0:c1], ACTF.Exp, bias=bias_c[:])

        def exp_b(k):
            c0, c1 = k * 1024, min((k + 1) * 1024, CB)
            nc.scalar.activation(
                exe_b[:, c0:c1], raw_all[:, CF + c0 : CF + c1], ACTF.Exp, bias=bias_c[:]
            )

        exp_f(0)
        exp_b(0)

        # init states
        nc.vector.tensor_copy(a_hist[:, 0:BL], exe_f[:, 0:BL])
        nc.vector.memset(w_hist[:, 0:BL], 1.0)

        # ---------------- the two scans, interleaved ----------------
        for s in range(1, M + 1):
            if s in (2, 18, 34, 50):
                i = (s - 2) // 16 + 1
                dma_f(i)
                dma_b(i)
            if s == 66:
                emit_tables()
            if s % 32 == 8:
                k = s // 32 + 1
                if k * 1024 < CF:
                    exp_f(k)
            if s % 32 == 24:
                k = s // 32 + 1
                if k * 1024 < CB:
                    exp_b(k)

            # fwd step t=s:  a_s = exe_f[s] * (E^T a_{s-1})
            up_f = ps_f.tile([K, BL], f32, tag="up_f")
            nc.tensor.matmul(
                up_f[:], e_bf[:], a_hist[:, (s - 1) * BL : s * BL], start=True, stop=True
            )
            nc.vector.tensor_mul(
                a_hist[:, s * BL : (s + 1) * BL], up_f[:], exe_f[:, s * BL : (s + 1) * BL]
            )

            # bwd step j=s:  w_s = exe_b[s-1] * (E w_{s-1})
            if s <= JMAX:
                up_b = ps_b.tile([K, BL], f32, tag="up_b")
                nc.tensor.matmul(
                    up_b[:], et_bf[:], w_hist[:, (s - 1) * BL : s * BL], start=True, stop=True
                )
                nc.vector.tensor_mul(
                    w_hist[:, s * BL : (s + 1) * BL], up_b[:], exe_b[:, (s - 1) * BL : s * BL]
                )

        # ---------------- gold score (gathers run during the scan) ----------------
        # unary: one bucketed gather from raw_all + masked accumulation
        nc.gpsimd.ap_gather(
            gu[:], raw_all[:], idx_u[:, :], channels=128,
            num_elems=(CF + CB) // 2, d=2, num_idxs=NIU,
        )
        nc.vector.scalar_tensor_tensor(
            junk[:], gu[:], 1.0, mu_all[:], OP.mult, OP.mult,
            accum_out=u_acc[:, 0:1],
        )
        # pair: trans contracted against the host-computed transition-count
        # matrix C (labels and mask are host-known): pair_tot = <C, trans>.
        pair_acc = small_pool.tile([128, 1], f32, tag="pair_acc")
        pair_junk = small_pool.tile([128, K], f32, tag="pair_junk")
        nc.vector.scalar_tensor_tensor(
            pair_junk[:], cmat[:], 1.0, trs[:], OP.mult, OP.mult,
            accum_out=pair_acc[:],
        )
        # score_tot = sum_p(u_acc + pair_acc) via PE column sum
        ones_f = small_pool.tile([128, 1], f32, tag="ones_f")
        nc.vector.memset(ones_f[:], 1.0)
        sc_ps = ps_misc.tile([1, 1], f32, tag="mm_fin")
        nc.tensor.matmul(sc_ps[:], ones_f[:], u_acc[:], start=True, stop=False)
        nc.tensor.matmul(sc_ps[:], ones_f[:], pair_acc[:], start=False, stop=True)
        score_tot = small_pool.tile([1, 1], f32, tag="score_tot")
        nc.vector.tensor_copy(score_tot[:], sc_ps[:])

        # ---------------- capture + logZ + loss ----------------
        nc.gpsimd.ap_gather(
            ga[:], a_hist[:], idx_cap[:, 0:2], channels=128,
            num_elems=CF // 2, d=2, num_idxs=32,
        )
        nc.gpsimd.ap_gather(
            gw[:], w_hist[:], idx_cap[:, 2:4], channels=128,
            num_elems=CW // 2, d=2, num_idxs=32,
        )
        nc.vector.tensor_mul(prod[:], ga[:], gw[:])
        dots_ev = ps_misc.tile([1, 16], f32, tag="mm_ev")
        nc.tensor.matmul(dots_ev[:], ones_col[:], prod[:, 0:64:4], start=True, stop=True)
        dots_od = ps_misc.tile([1, 16], f32, tag="mm_od")
        nc.tensor.matmul(dots_od[:], ones_col[:], prod[:, 3:64:4], start=True, stop=True)
        nc.vector.tensor_copy(dots[:, 0:BL:2], dots_ev[:])
        nc.vector.tensor_copy(dots[:, 1:BL:2], dots_od[:])
        nc.scalar.activation(ln_row[:], dots[:], ACTF.Ln)
        # lc = ln(dot) + c*L
        nc.vector.scalar_tensor_tensor(
            lc_row[:], seqf[:], C_LOG, ln_row[:], OP.mult, OP.add
        )
        nc.vector.tensor_reduce(t1[:], lc_row[:], AX.X, OP.add)
        nc.vector.tensor_sub(loss_sb[:], t1[:], score_tot[:])
        nc.sync.dma_start(loss_d[:, :], loss_sb[:])

    nc.compile()
    return nc


def _get_program():
    if "prog" not in _CACHE:
        _CACHE["prog"] = _build_program()
    return _CACHE["prog"]


def _core_tables(lgT_bf, lab, L):
    """Per-core tables: raw_all layout + gather indices/masks.

    lgT_bf: [K, T, BL] bf16 transposed logits, lab: [BL, T] int32, L: [BL]."""
    import ml_dtypes

    bf = ml_dtypes.bfloat16
    t = {}
    # raw_all: fwd t=0..M, then bwd j=1..JMAX time-reversed per row
    raw_f = lgT_bf[:, : M + 1, :].reshape(128, -1)
    tidx = np.maximum(L[None, :] - np.arange(1, JMAX + 1)[:, None], 0)  # [j, b]
    raw_b = lgT_bf[:, tidx, np.arange(BL)[None, :]].reshape(128, -1)
    t["raw_all"] = np.ascontiguousarray(
        np.concatenate([raw_f, raw_b], axis=1), dtype=bf
    )

    # capture indices (d=2 units): slot i=b lives at idx-col (c= b//16, pp=b%16)
    p = np.arange(128)[:, None]
    cgrid = np.arange(2)[None, :]
    bcap = cgrid * 16 + (p % 16)
    ta = np.minimum(L - 1, M)
    jw = np.maximum(L - 1 - M, 0)
    idx_a = (ta[bcap] * 16 + bcap // 2).astype(np.int16)
    idx_w = (jw[bcap] * 16 + bcap // 2).astype(np.int16)
    t["idx_cap"] = np.concatenate([idx_a, idx_w], axis=1)

    # pair: transition-count matrix C[i,j] = #{(b,t): lab=i->j, t+1 < L_b}
    act = (np.arange(T - 1)[None, :] + 1) < L[:, None]
    i_lab = lab[:, :-1][act]
    j_lab = lab[:, 1:][act]
    cmat = np.zeros((K, K), np.float32)
    np.add.at(cmat, (i_lab, j_lab), 1.0)
    t["cmat"] = cmat

    # unary: bucket active (b,t) entries by label's gpsimd core
    bb, tt = np.nonzero(np.arange(T)[None, :] < L[:, None])
    kk = lab[bb, tt]
    fwd_side = tt <= M
    unit = np.where(
        fwd_side, tt * 16 + bb // 2, (M + (L[bb] - tt)) * 16 + bb // 2
    ).astype(np.int64)
    par = (bb & 1).astype(np.int64)
    core = kk >> 4
    owner = kk & 15
    order = np.argsort(core, kind="stable")
    core_s, unit_s, owner_s, par_s = core[order], unit[order], owner[order], par[order]
    counts = np.bincount(core_s, minlength=8)
    assert counts.max() <= NIU, f"unary bucket overflow: {counts.max()}"
    idx_flat = np.zeros((8, NIU), np.int64)
    own_flat = np.full((8, NIU), -1, np.int64)
    par_flat = np.zeros((8, NIU), np.int64)
    off = 0
    for g in range(8):
        n = counts[g]
        idx_flat[g, :n] = unit_s[off : off + n]
        own_flat[g, :n] = owner_s[off : off + n]
        par_flat[g, :n] = par_s[off : off + n]
        off += n
    idx_u = np.zeros((128, NIU // 16), np.int16)
    s = np.arange(NIU)
    for g in range(8):
        idx_u[16 * g + (s % 16), s // 16] = idx_flat[g].astype(np.int16)
    t["idx_u"] = idx_u
    pp16 = np.arange(16)
    mu_all = np.zeros((128, 2 * NIU), np.float32)
    for g in range(8):
        own_match = own_flat[g][None, :] == pp16[:, None]  # [16, NIU]
        mu_all[16 * g : 16 * g + 16, 0::2] = own_match & (par_flat[g][None, :] == 0)
        mu_all[16 * g : 16 * g + 16, 1::2] = own_match & (par_flat[g][None, :] == 1)
    t["mu_all"] = mu_all.astype(bf)
    return t


def _make_in_maps(logits, labels, seq_lens, trans):
    import ml_dtypes

    bf = ml_dtypes.bfloat16
    logits = np.asarray(logits, dtype=np.float32)
    labels = np.asarray(labels, dtype=np.int64)
    seq_lens = np.asarray(seq_lens, dtype=np.int64)
    trans = np.asarray(trans, dtype=np.float32)
    transT = np.ascontiguousarray(trans.T)

    in_maps = []
    for c in range(NCORES):
        sl = slice(c * BL, (c + 1) * BL)
        lgT_bf = logits[sl].transpose(2, 1, 0).astype(bf)  # [K, T, BL]
        L = seq_lens[sl]
        m = {
            "trans": trans,
            "transT": transT,
            "seqf_row": L.astype(np.float32).reshape(1, BL),
        }
        m.update(_core_tables(lgT_bf, labels[sl], L))
        in_maps.append(m)
    return in_maps


def kernel(logits, labels, seq_lens, trans):
    from concourse.bass_utils import run_bass_kernel_spmd

    nc = _get_program()
    in_maps = _make_in_maps(logits, labels, seq_lens, trans)
    res = run_bass_kernel_spmd(nc, in_maps, list(range(NCORES)))
    total = sum(float(res.results[c]["loss"][0, 0]) for c in range(NCORES))
    return np.float32(total)
